# revision 3
# baseline (speedup 1.0000x reference)
"""Trainium2 Bass kernel for nn_KineticModel (gnn_message_passing), v3.

Same math and matmul structure as v2 (see kernel_v2.py), but the two S
operand streams are shipped NIBBLE-PACKED (two entries per byte) and
decoded on-chip by the vector engine:

  * s_sub (= relu(-S), values {0,1,2}, species-major) and s_t (= S,
    values {-2..2} stored +2 -> {0..4}, reaction-major) each pack two
    columns per byte: byte i of a 2048-wide tile row holds
    (col i) in the high nibble and (col i + 1024) in the low nibble.
  * decode is one DVE tensor_scalar per half-tile:
      hi: (b >> 4)  [- 2]      lo: (b & 15)  [- 2]
    writing contiguous fp8 halves; fp8 values are exact integers, so
    precision is identical to v2 (rel err ~3e-3).
  * total shipped S bytes: 16.8 MB/core vs 33.4 (v2) / 67 (v1).
"""

import sys

if "/opt/trn_rl_repo" not in sys.path:
    sys.path.insert(0, "/opt/trn_rl_repo")

import numpy as np
import ml_dtypes

import concourse.bacc as bacc
import concourse.mybir as mybir
from concourse.tile import TileContext
from concourse.bass_utils import run_bass_kernel_spmd

F32 = mybir.dt.float32
FP8 = mybir.dt.float8e4
U8 = mybir.dt.uint8
FP8_NP = ml_dtypes.float8_e4m3

N_SPECIES = 8192
N_RXN = 16384
N_BAL = 7680
N_CORES = 8
R_CORE = N_RXN // N_CORES        # 2048 reactions per core
SB = N_SPECIES // 128            # 64 species blocks
RB = R_CORE // 128               # 16 reaction blocks per core
NQ = 4                           # species quarters for matvec2 psum
QS = N_SPECIES // NQ             # 2048 species per quarter
HC = R_CORE // 2                 # packed bytes per s_sub tile row (1024)
HQ = QS // 2                     # packed bytes per s_t tile row (1024)

_CACHE = {}


def _build_nc():
    nc = bacc.Bacc(None, target_bir_lowering=False, debug=False)
    # nibble-packed relu(-S) species-major:
    #   byte [sb, p, i] = relu_val(col i) << 4 | relu_val(col i+1024)
    s_sub = nc.declare_dram_parameter("s_sub", [SB, 128, HC], U8, isOutput=False)
    # nibble-packed (S+2) reaction-major:
    #   byte [q, j, p, i] = (S+2)(s' = i) << 4 | (S+2)(s' = i+1024)
    s_t = nc.declare_dram_parameter("s_t", [NQ, RB, 128, HQ], U8, isOutput=False)
    xa = nc.declare_dram_parameter("xa", [128, SB], F32, isOutput=False)
    xb = nc.declare_dram_parameter("xb", [128, SB], F32, isOutput=False)
    kcat = nc.declare_dram_parameter("kcat", [1, R_CORE], F32, isOutput=False)
    out = nc.declare_dram_parameter("out", [2, N_SPECIES], F32, isOutput=True)

    ts = mybir.AluOpType
    with TileContext(nc) as tc:
        with (
            tc.tile_pool(name="small", bufs=1) as small,
            tc.tile_pool(name="ssubp", bufs=6) as ssubp_pool,
            tc.tile_pool(name="ssub", bufs=4) as ssub_pool,
            tc.tile_pool(name="stp", bufs=6) as stp_pool,
            tc.tile_pool(name="st", bufs=4) as st_pool,
            tc.tile_pool(name="stage", bufs=2) as stage_pool,
            tc.tile_pool(name="psv", bufs=1, space="PSUM") as psv_pool,
            tc.tile_pool(name="psd", bufs=1, space="PSUM") as psd_pool,
        ):
            # ---- logc = Ln(xa) + xb, split into interleaved hi/lo fp8 ----
            xa_t = small.tile([128, SB], F32, tag="xa")
            xb_t = small.tile([128, SB], F32, tag="xb")
            kcat_t = small.tile([1, R_CORE], F32, tag="kcat")
            nc.sync.dma_start(out=xa_t, in_=xa[:])
            nc.sync.dma_start(out=xb_t, in_=xb[:])
            nc.sync.dma_start(out=kcat_t, in_=kcat[:])

            lg = small.tile([128, SB], F32, tag="lg")
            nc.scalar.activation(lg, xa_t, mybir.ActivationFunctionType.Ln)
            logc = small.tile([128, SB], F32, tag="logc")
            nc.vector.tensor_tensor(out=logc, in0=lg, in1=xb_t, op=ts.add)

            logc_hl = small.tile([128, 2 * SB], FP8, tag="logc_hl")
            nc.vector.tensor_copy(out=logc_hl[:, 0 : 2 * SB : 2], in_=logc)
            lh_f = small.tile([128, SB], F32, tag="lh_f")
            nc.vector.tensor_copy(out=lh_f, in_=logc_hl[:, 0 : 2 * SB : 2])
            nc.vector.tensor_tensor(
                out=logc_hl[:, 1 : 2 * SB : 2], in0=logc, in1=lh_f, op=ts.subtract
            )

            # ---- matvec1 ----
            psum_v = psv_pool.tile([2, R_CORE], F32, tag="psum_v")
            for sb in range(SB):
                pt = ssubp_pool.tile([128, HC], U8, tag="ssubp")
                nc.sync.dma_start(out=pt, in_=s_sub[sb])
                au = ssub_pool.tile([128, R_CORE], U8, tag="ssub_u")
                nc.vector.tensor_scalar(
                    out=au[:, 0:HC], in0=pt, scalar1=4, scalar2=0,
                    op0=ts.logical_shift_right, op1=ts.bypass,
                )
                nc.vector.tensor_scalar(
                    out=au[:, HC:R_CORE], in0=pt, scalar1=15, scalar2=0,
                    op0=ts.bitwise_and, op1=ts.bypass,
                )
                at = ssub_pool.tile([128, R_CORE], FP8, tag="ssub")
                nc.vector.tensor_copy(out=at, in_=au)
                for rc in range(R_CORE // 512):
                    nc.tensor.matmul(
                        psum_v[:, rc * 512 : (rc + 1) * 512],
                        logc_hl[:, 2 * sb : 2 * sb + 2],
                        at[:, rc * 512 : (rc + 1) * 512],
                        start=(sb == 0),
                        stop=(sb == SB - 1),
                        skip_group_check=True,
                    )

            # ---- v = exp(hi_row + lo_row + kcat), hi/lo fp8 split ----
            pv = small.tile([2, R_CORE], F32, tag="pv")
            nc.vector.tensor_copy(out=pv, in_=psum_v)
            pvf = small.tile([1, 2 * R_CORE], F32, tag="pvf")
            nc.sync.dma_start(out=pvf[:, 0:R_CORE], in_=pv[0:1, :])
            nc.sync.dma_start(out=pvf[:, R_CORE : 2 * R_CORE], in_=pv[1:2, :])
            lv = small.tile([1, R_CORE], F32, tag="lv")
            nc.vector.tensor_tensor(
                out=lv, in0=pvf[:, 0:R_CORE], in1=pvf[:, R_CORE : 2 * R_CORE],
                op=ts.add,
            )
            lvk = small.tile([1, R_CORE], F32, tag="lvk")
            nc.vector.tensor_tensor(out=lvk, in0=lv, in1=kcat_t, op=ts.add)
            v_f = small.tile([1, R_CORE], F32, tag="v_f")
            nc.scalar.activation(v_f, lvk, mybir.ActivationFunctionType.Exp)

            vscr = nc.dram_tensor("vscr", [1, R_CORE], F32)
            nc.sync.dma_start(out=vscr[:], in_=v_f)
            v_pm = small.tile([128, RB], F32, tag="v_pm")
            nc.sync.dma_start(out=v_pm, in_=vscr.reshape((128, RB))[:])

            v_hl = small.tile([128, 2 * RB], FP8, tag="v_hl")
            nc.vector.tensor_copy(out=v_hl[:, 0 : 2 * RB : 2], in_=v_pm)
            vh_f = small.tile([128, RB], F32, tag="vh_f")
            nc.vector.tensor_copy(out=vh_f, in_=v_hl[:, 0 : 2 * RB : 2])
            nc.vector.tensor_tensor(
                out=v_hl[:, 1 : 2 * RB : 2], in0=v_pm, in1=vh_f, op=ts.subtract
            )

            # ---- matvec2 ships S+2 (unsigned nibbles), so it computes
            # (S+2) @ v_parts = S@v_parts + 2*sum(v_parts) * ones.  Return
            # the exact fp8 v parts (as f32) so the host can subtract the
            # scalar correction 2*sum(v_hi + v_lo) exactly.
            out_v = nc.declare_dram_parameter("out_v", [128, 2 * RB], F32,
                                              isOutput=True)
            v_parts = small.tile([128, 2 * RB], F32, tag="v_parts")
            nc.vector.tensor_copy(out=v_parts[:, 0:RB], in_=vh_f)
            nc.vector.tensor_copy(
                out=v_parts[:, RB : 2 * RB], in_=v_hl[:, 1 : 2 * RB : 2]
            )
            nc.sync.dma_start(out=out_v[:], in_=v_parts)

            for q in range(NQ):
                psum_dc = psd_pool.tile([2, QS], F32, tag="psum_dc")
                for j in range(RB):
                    ptb = stp_pool.tile([128, HQ], U8, tag="stp")
                    nc.sync.dma_start(out=ptb, in_=s_t[q, j])
                    bu = st_pool.tile([128, QS], U8, tag="st_u")
                    nc.vector.tensor_scalar(
                        out=bu[:, 0:HQ], in0=ptb, scalar1=4, scalar2=0,
                        op0=ts.logical_shift_right, op1=ts.bypass,
                    )
                    nc.vector.tensor_scalar(
                        out=bu[:, HQ:QS], in0=ptb, scalar1=15, scalar2=0,
                        op0=ts.bitwise_and, op1=ts.bypass,
                    )
                    bt = st_pool.tile([128, QS], FP8, tag="st")
                    nc.vector.tensor_copy(out=bt, in_=bu)
                    for sc in range(QS // 512):
                        nc.tensor.matmul(
                            psum_dc[:, sc * 512 : (sc + 1) * 512],
                            v_hl[:, 2 * j : 2 * j + 2],
                            bt[:, sc * 512 : (sc + 1) * 512],
                            start=(j == 0),
                            stop=(j == RB - 1),
                            skip_group_check=True,
                        )
                st_out = stage_pool.tile([2, QS], F32, tag="stage")
                nc.vector.tensor_copy(out=st_out, in_=psum_dc)
                nc.sync.dma_start(out=out[:, q * QS : (q + 1) * QS], in_=st_out)
    nc.compile()
    return nc


def _prep_inputs(conc_balanced, S, balanced_species, unbalanced_species,
                 log_conc_unbalanced, log_kcat):
    """Host-side shard + layout prep (pure data movement / dtype casts)."""
    in_maps = []
    xa_full = np.ones(N_SPECIES, dtype=np.float32)
    xb_full = np.zeros(N_SPECIES, dtype=np.float32)
    xa_full[np.asarray(balanced_species)] = np.asarray(conc_balanced)
    xb_full[np.asarray(unbalanced_species)] = np.asarray(log_conc_unbalanced)
    xa_pm = np.ascontiguousarray(xa_full.reshape(SB, 128).T)
    xb_pm = np.ascontiguousarray(xb_full.reshape(SB, 128).T)

    S = np.asarray(S)
    log_kcat = np.asarray(log_kcat)
    for c in range(N_CORES):
        r0 = c * R_CORE
        sl = S[:, r0 : r0 + R_CORE]                          # [8192, 2048] f32
        sub = np.maximum(-sl, 0.0).astype(np.uint8).reshape(SB, 128, R_CORE)
        s_sub = (sub[:, :, 0:HC] << 4) | sub[:, :, HC:R_CORE]
        # s_t byte [q, j, p, i]: S+2 at (s' = i | i+HQ, r = r0 + p*16 + j)
        sp2 = (sl + 2.0).astype(np.uint8).T                  # [2048, 8192]
        sp2 = sp2.reshape(128, RB, NQ, QS).transpose(2, 1, 0, 3)
        s_t = (sp2[..., 0:HQ] << 4) | sp2[..., HQ:QS]
        kcat_pm = log_kcat[r0 : r0 + R_CORE].astype(np.float32).reshape(1, R_CORE)
        in_maps.append(
            {
                "s_sub": np.ascontiguousarray(s_sub),
                "s_t": np.ascontiguousarray(s_t),
                "xa": xa_pm,
                "xb": xb_pm,
                "kcat": np.ascontiguousarray(kcat_pm),
            }
        )
    return in_maps


def kernel(**inputs) -> np.ndarray:
    if "nc" not in _CACHE:
        _CACHE["nc"] = _build_nc()
    nc = _CACHE["nc"]
    in_maps = _prep_inputs(**inputs)
    res = run_bass_kernel_spmd(nc, in_maps, core_ids=list(range(N_CORES)))
    acc = np.zeros(N_SPECIES, dtype=np.float64)
    for c in range(N_CORES):
        o = res.results[c]["out"].astype(np.float64)     # [2, 8192], (S+2)@v
        vp = res.results[c]["out_v"].astype(np.float64)  # [128, 2*RB] v parts
        corr = 2.0 * vp.sum()
        acc += o[0] + o[1] - corr
    return acc[:N_BAL].astype(np.float32)


# revision 4
# speedup vs baseline: 1.1995x; 1.1995x over previous
"""Trainium2 Bass kernel for nn_KineticModel (gnn_message_passing), v4.

Same math and matmul structure as v2/v3, with both S operand streams
bit-packed and decoded on-chip by the vector engine:

  * s_sub (= relu(-S), values {0,1,2}, species-major) packs FOUR columns
    per byte (2 bits each): byte i of a row holds reactions
    {i, i+512, i+1024, i+1536}; decoded quarter q ((b >> 2q) & 3) lands
    in columns [512q, 512q+512) — exactly one matmul N-chunk.
  * s_t (= S+2, values {0..4}, reaction-major) packs two columns per
    byte (high/low nibble), as in v3.
  * decode is u8->u8 tensor_scalar (shift/and) + one u8->fp8 copy per
    tile (the direct u8->fp8 bitwise path crashes walrus); fp8 values
    are exact integers, so precision matches v2 (rel err ~3e-3).
  * total shipped S bytes: 12.6 MB/core vs 16.8 (v3) / 33.4 (v2) / 67 (v1).
"""

import sys

if "/opt/trn_rl_repo" not in sys.path:
    sys.path.insert(0, "/opt/trn_rl_repo")

import numpy as np
import ml_dtypes

import concourse.bacc as bacc
import concourse.mybir as mybir
from concourse.tile import TileContext
from concourse.bass_utils import run_bass_kernel_spmd

F32 = mybir.dt.float32
FP8 = mybir.dt.float8e4
U8 = mybir.dt.uint8
FP8_NP = ml_dtypes.float8_e4m3

N_SPECIES = 8192
N_RXN = 16384
N_BAL = 7680
N_CORES = 8
R_CORE = N_RXN // N_CORES        # 2048 reactions per core
SB = N_SPECIES // 128            # 64 species blocks
RB = R_CORE // 128               # 16 reaction blocks per core
NQ = 4                           # species quarters for matvec2 psum
QS = N_SPECIES // NQ             # 2048 species per quarter
PC = R_CORE // 4                 # packed bytes per s_sub tile row (512)
HQ = QS // 2                     # packed bytes per s_t tile row (1024)

_CACHE = {}


def _build_nc():
    nc = bacc.Bacc(None, target_bir_lowering=False, debug=False)
    # 2-bit-packed relu(-S) species-major:
    #   byte [sb, p, i] bits 2q:2q+2 = relu_val(col i + 512*q)
    s_sub = nc.declare_dram_parameter("s_sub", [SB, 128, PC], U8, isOutput=False)
    # nibble-packed (S+2) reaction-major:
    #   byte [q, j, p, i] = (S+2)(s' = i) << 4 | (S+2)(s' = i+1024)
    s_t = nc.declare_dram_parameter("s_t", [NQ, RB, 128, HQ], U8, isOutput=False)
    xa = nc.declare_dram_parameter("xa", [128, SB], F32, isOutput=False)
    xb = nc.declare_dram_parameter("xb", [128, SB], F32, isOutput=False)
    kcat = nc.declare_dram_parameter("kcat", [1, R_CORE], F32, isOutput=False)
    out = nc.declare_dram_parameter("out", [2, N_SPECIES], F32, isOutput=True)

    ts = mybir.AluOpType
    with TileContext(nc) as tc:
        with (
            tc.tile_pool(name="small", bufs=1) as small,
            tc.tile_pool(name="ssubp", bufs=6) as ssubp_pool,
            tc.tile_pool(name="ssub", bufs=4) as ssub_pool,
            tc.tile_pool(name="stp", bufs=6) as stp_pool,
            tc.tile_pool(name="st", bufs=4) as st_pool,
            tc.tile_pool(name="stage", bufs=2) as stage_pool,
            tc.tile_pool(name="psv", bufs=1, space="PSUM") as psv_pool,
            tc.tile_pool(name="psd", bufs=1, space="PSUM") as psd_pool,
        ):
            # ---- logc = Ln(xa) + xb, split into interleaved hi/lo fp8 ----
            xa_t = small.tile([128, SB], F32, tag="xa")
            xb_t = small.tile([128, SB], F32, tag="xb")
            kcat_t = small.tile([1, R_CORE], F32, tag="kcat")
            nc.sync.dma_start(out=xa_t, in_=xa[:])
            nc.sync.dma_start(out=xb_t, in_=xb[:])
            nc.sync.dma_start(out=kcat_t, in_=kcat[:])

            lg = small.tile([128, SB], F32, tag="lg")
            nc.scalar.activation(lg, xa_t, mybir.ActivationFunctionType.Ln)
            logc = small.tile([128, SB], F32, tag="logc")
            nc.vector.tensor_tensor(out=logc, in0=lg, in1=xb_t, op=ts.add)

            logc_hl = small.tile([128, 2 * SB], FP8, tag="logc_hl")
            nc.vector.tensor_copy(out=logc_hl[:, 0 : 2 * SB : 2], in_=logc)
            lh_f = small.tile([128, SB], F32, tag="lh_f")
            nc.vector.tensor_copy(out=lh_f, in_=logc_hl[:, 0 : 2 * SB : 2])
            nc.vector.tensor_tensor(
                out=logc_hl[:, 1 : 2 * SB : 2], in0=logc, in1=lh_f, op=ts.subtract
            )

            # ---- matvec1 ----
            psum_v = psv_pool.tile([2, R_CORE], F32, tag="psum_v")
            for sb in range(SB):
                pt = ssubp_pool.tile([128, PC], U8, tag="ssubp")
                nc.sync.dma_start(out=pt, in_=s_sub[sb])
                au = ssub_pool.tile([128, R_CORE], U8, tag="ssub_u")
                nc.vector.tensor_scalar(
                    out=au[:, 0:PC], in0=pt, scalar1=3, scalar2=0,
                    op0=ts.bitwise_and, op1=ts.bypass,
                )
                for qq in range(1, 4):
                    nc.vector.tensor_scalar(
                        out=au[:, qq * PC : (qq + 1) * PC], in0=pt,
                        scalar1=2 * qq, scalar2=3,
                        op0=ts.logical_shift_right, op1=ts.bitwise_and,
                    )
                at = ssub_pool.tile([128, R_CORE], FP8, tag="ssub")
                nc.vector.tensor_copy(out=at, in_=au)
                for rc in range(R_CORE // 512):
                    nc.tensor.matmul(
                        psum_v[:, rc * 512 : (rc + 1) * 512],
                        logc_hl[:, 2 * sb : 2 * sb + 2],
                        at[:, rc * 512 : (rc + 1) * 512],
                        start=(sb == 0),
                        stop=(sb == SB - 1),
                        skip_group_check=True,
                    )

            # ---- v = exp(hi_row + lo_row + kcat), hi/lo fp8 split ----
            pv = small.tile([2, R_CORE], F32, tag="pv")
            nc.vector.tensor_copy(out=pv, in_=psum_v)
            pvf = small.tile([1, 2 * R_CORE], F32, tag="pvf")
            nc.sync.dma_start(out=pvf[:, 0:R_CORE], in_=pv[0:1, :])
            nc.sync.dma_start(out=pvf[:, R_CORE : 2 * R_CORE], in_=pv[1:2, :])
            lv = small.tile([1, R_CORE], F32, tag="lv")
            nc.vector.tensor_tensor(
                out=lv, in0=pvf[:, 0:R_CORE], in1=pvf[:, R_CORE : 2 * R_CORE],
                op=ts.add,
            )
            lvk = small.tile([1, R_CORE], F32, tag="lvk")
            nc.vector.tensor_tensor(out=lvk, in0=lv, in1=kcat_t, op=ts.add)
            v_f = small.tile([1, R_CORE], F32, tag="v_f")
            nc.scalar.activation(v_f, lvk, mybir.ActivationFunctionType.Exp)

            vscr = nc.dram_tensor("vscr", [1, R_CORE], F32)
            nc.sync.dma_start(out=vscr[:], in_=v_f)
            v_pm = small.tile([128, RB], F32, tag="v_pm")
            nc.sync.dma_start(out=v_pm, in_=vscr.reshape((128, RB))[:])

            v_hl = small.tile([128, 2 * RB], FP8, tag="v_hl")
            nc.vector.tensor_copy(out=v_hl[:, 0 : 2 * RB : 2], in_=v_pm)
            vh_f = small.tile([128, RB], F32, tag="vh_f")
            nc.vector.tensor_copy(out=vh_f, in_=v_hl[:, 0 : 2 * RB : 2])
            nc.vector.tensor_tensor(
                out=v_hl[:, 1 : 2 * RB : 2], in0=v_pm, in1=vh_f, op=ts.subtract
            )

            # ---- matvec2 ships S+2 (unsigned nibbles), so it computes
            # (S+2) @ v_parts = S@v_parts + 2*sum(v_parts) * ones.  Return
            # the exact fp8 v parts (as f32) so the host can subtract the
            # scalar correction 2*sum(v_hi + v_lo) exactly.
            out_v = nc.declare_dram_parameter("out_v", [128, 2 * RB], F32,
                                              isOutput=True)
            v_parts = small.tile([128, 2 * RB], F32, tag="v_parts")
            nc.vector.tensor_copy(out=v_parts[:, 0:RB], in_=vh_f)
            nc.vector.tensor_copy(
                out=v_parts[:, RB : 2 * RB], in_=v_hl[:, 1 : 2 * RB : 2]
            )
            nc.sync.dma_start(out=out_v[:], in_=v_parts)

            for q in range(NQ):
                psum_dc = psd_pool.tile([2, QS], F32, tag="psum_dc")
                for j in range(RB):
                    ptb = stp_pool.tile([128, HQ], U8, tag="stp")
                    nc.sync.dma_start(out=ptb, in_=s_t[q, j])
                    bu = st_pool.tile([128, QS], U8, tag="st_u")
                    nc.vector.tensor_scalar(
                        out=bu[:, 0:HQ], in0=ptb, scalar1=4, scalar2=0,
                        op0=ts.logical_shift_right, op1=ts.bypass,
                    )
                    nc.vector.tensor_scalar(
                        out=bu[:, HQ:QS], in0=ptb, scalar1=15, scalar2=0,
                        op0=ts.bitwise_and, op1=ts.bypass,
                    )
                    bt = st_pool.tile([128, QS], FP8, tag="st")
                    nc.vector.tensor_copy(out=bt, in_=bu)
                    for sc in range(QS // 512):
                        nc.tensor.matmul(
                            psum_dc[:, sc * 512 : (sc + 1) * 512],
                            v_hl[:, 2 * j : 2 * j + 2],
                            bt[:, sc * 512 : (sc + 1) * 512],
                            start=(j == 0),
                            stop=(j == RB - 1),
                            skip_group_check=True,
                        )
                st_out = stage_pool.tile([2, QS], F32, tag="stage")
                nc.vector.tensor_copy(out=st_out, in_=psum_dc)
                nc.sync.dma_start(out=out[:, q * QS : (q + 1) * QS], in_=st_out)
    nc.compile()
    return nc


def _prep_inputs(conc_balanced, S, balanced_species, unbalanced_species,
                 log_conc_unbalanced, log_kcat):
    """Host-side shard + layout prep (pure data movement / dtype casts)."""
    in_maps = []
    xa_full = np.ones(N_SPECIES, dtype=np.float32)
    xb_full = np.zeros(N_SPECIES, dtype=np.float32)
    xa_full[np.asarray(balanced_species)] = np.asarray(conc_balanced)
    xb_full[np.asarray(unbalanced_species)] = np.asarray(log_conc_unbalanced)
    xa_pm = np.ascontiguousarray(xa_full.reshape(SB, 128).T)
    xb_pm = np.ascontiguousarray(xb_full.reshape(SB, 128).T)

    S = np.asarray(S)
    log_kcat = np.asarray(log_kcat)
    for c in range(N_CORES):
        r0 = c * R_CORE
        sl = S[:, r0 : r0 + R_CORE]                          # [8192, 2048] f32
        sub = np.maximum(-sl, 0.0).astype(np.uint8).reshape(SB, 128, R_CORE)
        s_sub = (
            sub[:, :, 0:PC]
            | (sub[:, :, PC : 2 * PC] << 2)
            | (sub[:, :, 2 * PC : 3 * PC] << 4)
            | (sub[:, :, 3 * PC : 4 * PC] << 6)
        )
        # s_t byte [q, j, p, i]: S+2 at (s' = i | i+HQ, r = r0 + p*16 + j)
        sp2 = (sl + 2.0).astype(np.uint8).T                  # [2048, 8192]
        sp2 = sp2.reshape(128, RB, NQ, QS).transpose(2, 1, 0, 3)
        s_t = (sp2[..., 0:HQ] << 4) | sp2[..., HQ:QS]
        kcat_pm = log_kcat[r0 : r0 + R_CORE].astype(np.float32).reshape(1, R_CORE)
        in_maps.append(
            {
                "s_sub": np.ascontiguousarray(s_sub),
                "s_t": np.ascontiguousarray(s_t),
                "xa": xa_pm,
                "xb": xb_pm,
                "kcat": np.ascontiguousarray(kcat_pm),
            }
        )
    return in_maps


def kernel(**inputs) -> np.ndarray:
    if "nc" not in _CACHE:
        _CACHE["nc"] = _build_nc()
    nc = _CACHE["nc"]
    in_maps = _prep_inputs(**inputs)
    res = run_bass_kernel_spmd(nc, in_maps, core_ids=list(range(N_CORES)))
    acc = np.zeros(N_SPECIES, dtype=np.float64)
    for c in range(N_CORES):
        o = res.results[c]["out"].astype(np.float64)     # [2, 8192], (S+2)@v
        vp = res.results[c]["out_v"].astype(np.float64)  # [128, 2*RB] v parts
        corr = 2.0 * vp.sum()
        acc += o[0] + o[1] - corr
    return acc[:N_BAL].astype(np.float32)


# revision 5
# speedup vs baseline: 1.6106x; 1.3427x over previous
"""Trainium2 Bass kernel for nn_KineticModel (gnn_message_passing), v5.

Same math as v2-v4, but S is shipped SPARSE (~1.6 MB/core instead of
12.6 MB dense-packed) and expanded on-chip by GPSIMD local_scatter:

  * S has ~214k nonzeros (density 0.16%).  Each matmul operand tile
    [128, 1024] is built by one local_scatter (zero-fills, then scatters
    that partition's (offset, value) pairs; -1 offsets are padding).
  * s_sub stream (relu(-S), species-major): idx/val pairs per
    (species row, 1024-reaction window), padded to 8 (measured max 7).
  * s_t stream (S, reaction-major, r = p*16 + j): idx/val pairs per
    (reaction row, 1024-species window), padded to 16 (measured max 11).
  * Tiles are bf16 (local_scatter needs 2-byte dtypes), so the hi/lo
    splits of logc and v are bf16 and overall rel err is ~4e-6 (v1
    level), with no unsigned-shift correction needed.
"""

import sys

if "/opt/trn_rl_repo" not in sys.path:
    sys.path.insert(0, "/opt/trn_rl_repo")

import numpy as np
import ml_dtypes

import concourse.bacc as bacc
import concourse.mybir as mybir
from concourse.tile import TileContext
from concourse.bass_utils import run_bass_kernel_spmd

F32 = mybir.dt.float32
BF16 = mybir.dt.bfloat16
I16 = mybir.dt.int16
BF16_NP = ml_dtypes.bfloat16

N_SPECIES = 8192
N_RXN = 16384
N_BAL = 7680
N_CORES = 8
R_CORE = N_RXN // N_CORES        # 2048 reactions per core
SB = N_SPECIES // 128            # 64 species blocks
RB = R_CORE // 128               # 16 reaction blocks per core
W = 1024                         # scatter tile width (num_elems)
NW1 = R_CORE // W                # 2 reaction windows (matvec1)
NW2 = N_SPECIES // W             # 8 species windows (matvec2)
NI1 = 8                          # idx pad for s_sub rows (max seen 7)
NI2 = 16                         # idx pad for s_t rows (max seen 11)

_CACHE = {}


def _build_nc():
    nc = bacc.Bacc(None, target_bir_lowering=False, debug=False)
    su_idx = nc.declare_dram_parameter("su_idx", [SB, NW1, 128, NI1], I16, isOutput=False)
    su_dat = nc.declare_dram_parameter("su_dat", [SB, NW1, 128, NI1], BF16, isOutput=False)
    st_idx = nc.declare_dram_parameter("st_idx", [NW2, RB, 128, NI2], I16, isOutput=False)
    st_dat = nc.declare_dram_parameter("st_dat", [NW2, RB, 128, NI2], BF16, isOutput=False)
    xa = nc.declare_dram_parameter("xa", [128, SB], F32, isOutput=False)
    xb = nc.declare_dram_parameter("xb", [128, SB], F32, isOutput=False)
    kcat = nc.declare_dram_parameter("kcat", [1, R_CORE], F32, isOutput=False)
    out = nc.declare_dram_parameter("out", [2, N_SPECIES], F32, isOutput=True)

    ts = mybir.AluOpType
    with TileContext(nc) as tc:
        with (
            tc.tile_pool(name="small", bufs=1) as small,
            tc.tile_pool(name="sui", bufs=6) as sui_pool,
            tc.tile_pool(name="sus", bufs=4) as sus_pool,
            tc.tile_pool(name="sti", bufs=6) as sti_pool,
            tc.tile_pool(name="sts", bufs=4) as sts_pool,
            tc.tile_pool(name="stage", bufs=2) as stage_pool,
            tc.tile_pool(name="psv", bufs=1, space="PSUM") as psv_pool,
            tc.tile_pool(name="psd", bufs=2, space="PSUM") as psd_pool,
        ):
            # ---- logc = Ln(xa) + xb, split into interleaved hi/lo bf16 ----
            xa_t = small.tile([128, SB], F32, tag="xa")
            xb_t = small.tile([128, SB], F32, tag="xb")
            kcat_t = small.tile([1, R_CORE], F32, tag="kcat")
            nc.sync.dma_start(out=xa_t, in_=xa[:])
            nc.sync.dma_start(out=xb_t, in_=xb[:])
            nc.sync.dma_start(out=kcat_t, in_=kcat[:])

            lg = small.tile([128, SB], F32, tag="lg")
            nc.scalar.activation(lg, xa_t, mybir.ActivationFunctionType.Ln)
            logc = small.tile([128, SB], F32, tag="logc")
            nc.vector.tensor_tensor(out=logc, in0=lg, in1=xb_t, op=ts.add)

            logc_hl = small.tile([128, 2 * SB], BF16, tag="logc_hl")
            nc.vector.tensor_copy(out=logc_hl[:, 0 : 2 * SB : 2], in_=logc)
            lh_f = small.tile([128, SB], F32, tag="lh_f")
            nc.vector.tensor_copy(out=lh_f, in_=logc_hl[:, 0 : 2 * SB : 2])
            nc.vector.tensor_tensor(
                out=logc_hl[:, 1 : 2 * SB : 2], in0=logc, in1=lh_f, op=ts.subtract
            )

            # ---- matvec1: psum_v += logc_hl[sb].T @ scatter(s_sub[sb, w]) ----
            psum_v = psv_pool.tile([2, R_CORE], F32, tag="psum_v")
            for sb in range(SB):
                for w in range(NW1):
                    it = sui_pool.tile([128, NI1], I16, tag="sui")
                    dt_ = sui_pool.tile([128, NI1], BF16, tag="sud")
                    nc.sync.dma_start(out=it, in_=su_idx[sb, w])
                    nc.sync.dma_start(out=dt_, in_=su_dat[sb, w])
                    sc = sus_pool.tile([128, W], BF16, tag="sus")
                    nc.gpsimd.local_scatter(
                        sc, dt_, it, channels=128, num_elems=W, num_idxs=NI1
                    )
                    for c in range(W // 512):
                        nc.tensor.matmul(
                            psum_v[:, w * W + c * 512 : w * W + (c + 1) * 512],
                            logc_hl[:, 2 * sb : 2 * sb + 2],
                            sc[:, c * 512 : (c + 1) * 512],
                            start=(sb == 0),
                            stop=(sb == SB - 1),
                            skip_group_check=True,
                        )

            # ---- v = exp(hi_row + lo_row + kcat), hi/lo bf16 split ----
            pv = small.tile([2, R_CORE], F32, tag="pv")
            nc.vector.tensor_copy(out=pv, in_=psum_v)
            pvf = small.tile([1, 2 * R_CORE], F32, tag="pvf")
            nc.sync.dma_start(out=pvf[:, 0:R_CORE], in_=pv[0:1, :])
            nc.sync.dma_start(out=pvf[:, R_CORE : 2 * R_CORE], in_=pv[1:2, :])
            lv = small.tile([1, R_CORE], F32, tag="lv")
            nc.vector.tensor_tensor(
                out=lv, in0=pvf[:, 0:R_CORE], in1=pvf[:, R_CORE : 2 * R_CORE],
                op=ts.add,
            )
            lvk = small.tile([1, R_CORE], F32, tag="lvk")
            nc.vector.tensor_tensor(out=lvk, in0=lv, in1=kcat_t, op=ts.add)
            v_f = small.tile([1, R_CORE], F32, tag="v_f")
            nc.scalar.activation(v_f, lvk, mybir.ActivationFunctionType.Exp)

            vscr = nc.dram_tensor("vscr", [1, R_CORE], F32)
            nc.sync.dma_start(out=vscr[:], in_=v_f)
            v_pm = small.tile([128, RB], F32, tag="v_pm")
            nc.sync.dma_start(out=v_pm, in_=vscr.reshape((128, RB))[:])

            v_hl = small.tile([128, 2 * RB], BF16, tag="v_hl")
            nc.vector.tensor_copy(out=v_hl[:, 0 : 2 * RB : 2], in_=v_pm)
            vh_f = small.tile([128, RB], F32, tag="vh_f")
            nc.vector.tensor_copy(out=vh_f, in_=v_hl[:, 0 : 2 * RB : 2])
            nc.vector.tensor_tensor(
                out=v_hl[:, 1 : 2 * RB : 2], in0=v_pm, in1=vh_f, op=ts.subtract
            )

            # ---- matvec2: psum_dc += v_hl[j].T @ scatter(s_t[w, j]) ----
            for w in range(NW2):
                psum_dc = psd_pool.tile([2, W], F32, tag="psum_dc")
                for j in range(RB):
                    it = sti_pool.tile([128, NI2], I16, tag="sti")
                    dt_ = sti_pool.tile([128, NI2], BF16, tag="std")
                    nc.sync.dma_start(out=it, in_=st_idx[w, j])
                    nc.sync.dma_start(out=dt_, in_=st_dat[w, j])
                    sc = sts_pool.tile([128, W], BF16, tag="sts")
                    nc.gpsimd.local_scatter(
                        sc, dt_, it, channels=128, num_elems=W, num_idxs=NI2
                    )
                    for c in range(W // 512):
                        nc.tensor.matmul(
                            psum_dc[:, c * 512 : (c + 1) * 512],
                            v_hl[:, 2 * j : 2 * j + 2],
                            sc[:, c * 512 : (c + 1) * 512],
                            start=(j == 0),
                            stop=(j == RB - 1),
                            skip_group_check=True,
                        )
                st_out = stage_pool.tile([2, W], F32, tag="stage")
                nc.vector.tensor_copy(out=st_out, in_=psum_dc)
                nc.sync.dma_start(out=out[:, w * W : (w + 1) * W], in_=st_out)
    nc.compile()
    return nc


def _sparse_rows(mat, n_rows, width, pad):
    """mat: [n_rows, width] -> (idx [n_rows, pad] i16, val [n_rows, pad] bf16).

    Row-wise nonzero offsets (pad with -1) and values.  Raises if any row
    has more than `pad` nonzeros.
    """
    idx = np.full((n_rows, pad), -1, np.int16)
    val = np.zeros((n_rows, pad), np.float32)
    rr, cc = np.nonzero(mat)
    if len(rr):
        # rank of each entry within its row
        order = np.lexsort((cc, rr))
        rr, cc = rr[order], cc[order]
        starts = np.searchsorted(rr, np.arange(n_rows))
        rank = np.arange(len(rr)) - starts[rr]
        if rank.max() >= pad:
            raise ValueError(f"row nnz {rank.max() + 1} exceeds pad {pad}")
        idx[rr, rank] = cc.astype(np.int16)
        val[rr, rank] = mat[rr, cc]
    return idx, val.astype(BF16_NP)


def _prep_inputs(conc_balanced, S, balanced_species, unbalanced_species,
                 log_conc_unbalanced, log_kcat):
    """Host-side shard + sparse-encode prep (pure data movement / casts)."""
    in_maps = []
    xa_full = np.ones(N_SPECIES, dtype=np.float32)
    xb_full = np.zeros(N_SPECIES, dtype=np.float32)
    xa_full[np.asarray(balanced_species)] = np.asarray(conc_balanced)
    xb_full[np.asarray(unbalanced_species)] = np.asarray(log_conc_unbalanced)
    xa_pm = np.ascontiguousarray(xa_full.reshape(SB, 128).T)
    xb_pm = np.ascontiguousarray(xb_full.reshape(SB, 128).T)

    S = np.asarray(S)
    log_kcat = np.asarray(log_kcat)
    for c in range(N_CORES):
        r0 = c * R_CORE
        sl = S[:, r0 : r0 + R_CORE].astype(np.float32)       # [8192, 2048]
        # matvec1 stream: rows = (sb, w, p) -> species sb*128+p, window w
        sub = np.maximum(-sl, 0.0)                           # [8192, 2048]
        sub_rows = sub.reshape(SB, 128, NW1, W).transpose(0, 2, 1, 3)
        i1, v1 = _sparse_rows(
            sub_rows.reshape(-1, W), SB * NW1 * 128, W, NI1
        )
        # matvec2 stream: rows = (w, j, p) -> reaction r = p*16 + j,
        # species window w
        slT = sl.T                                           # [2048, 8192]
        st_rows = slT.reshape(128, RB, NW2, W).transpose(2, 1, 0, 3)
        i2, v2 = _sparse_rows(
            st_rows.reshape(-1, W), NW2 * RB * 128, W, NI2
        )
        kcat_pm = log_kcat[r0 : r0 + R_CORE].astype(np.float32).reshape(1, R_CORE)
        in_maps.append(
            {
                "su_idx": np.ascontiguousarray(i1.reshape(SB, NW1, 128, NI1)),
                "su_dat": np.ascontiguousarray(v1.reshape(SB, NW1, 128, NI1)),
                "st_idx": np.ascontiguousarray(i2.reshape(NW2, RB, 128, NI2)),
                "st_dat": np.ascontiguousarray(v2.reshape(NW2, RB, 128, NI2)),
                "xa": xa_pm,
                "xb": xb_pm,
                "kcat": np.ascontiguousarray(kcat_pm),
            }
        )
    return in_maps


def kernel(**inputs) -> np.ndarray:
    if "nc" not in _CACHE:
        _CACHE["nc"] = _build_nc()
    nc = _CACHE["nc"]
    in_maps = _prep_inputs(**inputs)
    res = run_bass_kernel_spmd(nc, in_maps, core_ids=list(range(N_CORES)))
    acc = np.zeros(N_SPECIES, dtype=np.float64)
    for c in range(N_CORES):
        o = res.results[c]["out"]
        acc += o[0].astype(np.float64) + o[1].astype(np.float64)
    return acc[:N_BAL].astype(np.float32)


# revision 7
# speedup vs baseline: 1.7061x; 1.0593x over previous
"""Trainium2 Bass kernel for nn_KineticModel (gnn_message_passing), v5.

Same math as v2-v4, but S is shipped SPARSE (~1.6 MB/core instead of
12.6 MB dense-packed) and expanded on-chip by GPSIMD local_scatter:

  * S has ~214k nonzeros (density 0.16%).  Each matmul operand tile
    [128, 1024] is built by one local_scatter (zero-fills, then scatters
    that partition's (offset, value) pairs; -1 offsets are padding).
  * s_sub stream (relu(-S), species-major): idx/val pairs per
    (species row, 1024-reaction window), padded to 8 (measured max 7).
  * s_t stream (S, reaction-major, r = p*16 + j): idx/val pairs per
    (reaction row, 1024-species window), padded to 16 (measured max 11).
  * Tiles are bf16 (local_scatter needs 2-byte dtypes), so the hi/lo
    splits of logc and v are bf16 and overall rel err is ~4e-6 (v1
    level), with no unsigned-shift correction needed.
"""

import sys

if "/opt/trn_rl_repo" not in sys.path:
    sys.path.insert(0, "/opt/trn_rl_repo")

import numpy as np
import ml_dtypes

import concourse.bacc as bacc
import concourse.mybir as mybir
from concourse.tile import TileContext
from concourse.bass_utils import run_bass_kernel_spmd

F32 = mybir.dt.float32
BF16 = mybir.dt.bfloat16
I16 = mybir.dt.int16
BF16_NP = ml_dtypes.bfloat16

N_SPECIES = 8192
N_RXN = 16384
N_BAL = 7680
N_CORES = 8
R_CORE = N_RXN // N_CORES        # 2048 reactions per core
SB = N_SPECIES // 128            # 64 species blocks
RB = R_CORE // 128               # 16 reaction blocks per core
W = 1024                         # scatter tile width (num_elems)
NW1 = R_CORE // W                # 2 reaction windows (matvec1)
NW2 = N_SPECIES // W             # 8 species windows (matvec2)
NI1 = 8                          # idx pad for s_sub rows (max seen 7)
NI2 = 16                         # idx pad for s_t rows (max seen 11)

_CACHE = {}


def _build_nc():
    nc = bacc.Bacc(None, target_bir_lowering=False, debug=False)
    su_idx = nc.declare_dram_parameter("su_idx", [SB, NW1, 128, NI1], I16, isOutput=False)
    su_dat = nc.declare_dram_parameter("su_dat", [SB, NW1, 128, NI1], BF16, isOutput=False)
    st_idx = nc.declare_dram_parameter("st_idx", [NW2, RB, 128, NI2], I16, isOutput=False)
    st_dat = nc.declare_dram_parameter("st_dat", [NW2, RB, 128, NI2], BF16, isOutput=False)
    xa = nc.declare_dram_parameter("xa", [128, SB], F32, isOutput=False)
    xb = nc.declare_dram_parameter("xb", [128, SB], F32, isOutput=False)
    kcat = nc.declare_dram_parameter("kcat", [1, R_CORE], F32, isOutput=False)
    out = nc.declare_dram_parameter("out", [2, N_SPECIES], F32, isOutput=True)

    ts = mybir.AluOpType
    with TileContext(nc) as tc:
        with (
            tc.tile_pool(name="small", bufs=1) as small,
            tc.tile_pool(name="sui", bufs=6) as sui_pool,
            tc.tile_pool(name="sus", bufs=4) as sus_pool,
            tc.tile_pool(name="sti", bufs=6) as sti_pool,
            tc.tile_pool(name="sts", bufs=4) as sts_pool,
            tc.tile_pool(name="stage", bufs=2) as stage_pool,
            tc.tile_pool(name="psv", bufs=1, space="PSUM") as psv_pool,
            tc.tile_pool(name="psd", bufs=1, space="PSUM") as psd_pool,
        ):
            # ---- logc = Ln(xa) + xb, split into interleaved hi/lo bf16 ----
            xa_t = small.tile([128, SB], F32, tag="xa")
            xb_t = small.tile([128, SB], F32, tag="xb")
            kcat_t = small.tile([1, R_CORE], F32, tag="kcat")
            nc.sync.dma_start(out=xa_t, in_=xa[:])
            nc.sync.dma_start(out=xb_t, in_=xb[:])
            nc.sync.dma_start(out=kcat_t, in_=kcat[:])

            lg = small.tile([128, SB], F32, tag="lg")
            nc.scalar.activation(lg, xa_t, mybir.ActivationFunctionType.Ln)
            logc = small.tile([128, SB], F32, tag="logc")
            nc.vector.tensor_tensor(out=logc, in0=lg, in1=xb_t, op=ts.add)

            logc_hl = small.tile([128, 2 * SB], BF16, tag="logc_hl")
            nc.vector.tensor_copy(out=logc_hl[:, 0 : 2 * SB : 2], in_=logc)
            lh_f = small.tile([128, SB], F32, tag="lh_f")
            nc.vector.tensor_copy(out=lh_f, in_=logc_hl[:, 0 : 2 * SB : 2])
            nc.vector.tensor_tensor(
                out=logc_hl[:, 1 : 2 * SB : 2], in0=logc, in1=lh_f, op=ts.subtract
            )

            # ---- matvec1: psum_v += logc_hl[sb].T @ scatter(s_sub[sb, w]) ----
            psum_v = psv_pool.tile([2, R_CORE], F32, tag="psum_v")
            for sb in range(SB):
                for w in range(NW1):
                    it = sui_pool.tile([128, NI1], I16, tag="sui")
                    dt_ = sui_pool.tile([128, NI1], BF16, tag="sud")
                    nc.sync.dma_start(out=it, in_=su_idx[sb, w])
                    nc.sync.dma_start(out=dt_, in_=su_dat[sb, w])
                    sc = sus_pool.tile([128, W], BF16, tag="sus")
                    nc.gpsimd.local_scatter(
                        sc, dt_, it, channels=128, num_elems=W, num_idxs=NI1
                    )
                    # DVE staging copy: PE reads only DVE output, so a
                    # GPSIMD-write/PE-read sync hazard on sc cannot bite.
                    scs = sus_pool.tile([128, W], BF16, tag="sus_s")
                    nc.vector.tensor_copy(out=scs, in_=sc)
                    for c in range(W // 512):
                        nc.tensor.matmul(
                            psum_v[:, w * W + c * 512 : w * W + (c + 1) * 512],
                            logc_hl[:, 2 * sb : 2 * sb + 2],
                            scs[:, c * 512 : (c + 1) * 512],
                            start=(sb == 0),
                            stop=(sb == SB - 1),
                            skip_group_check=True,
                        )

            # ---- v = exp(hi_row + lo_row + kcat), hi/lo bf16 split ----
            pv = small.tile([2, R_CORE], F32, tag="pv")
            nc.vector.tensor_copy(out=pv, in_=psum_v)
            pvf = small.tile([1, 2 * R_CORE], F32, tag="pvf")
            nc.sync.dma_start(out=pvf[:, 0:R_CORE], in_=pv[0:1, :])
            nc.sync.dma_start(out=pvf[:, R_CORE : 2 * R_CORE], in_=pv[1:2, :])
            lv = small.tile([1, R_CORE], F32, tag="lv")
            nc.vector.tensor_tensor(
                out=lv, in0=pvf[:, 0:R_CORE], in1=pvf[:, R_CORE : 2 * R_CORE],
                op=ts.add,
            )
            lvk = small.tile([1, R_CORE], F32, tag="lvk")
            nc.vector.tensor_tensor(out=lvk, in0=lv, in1=kcat_t, op=ts.add)
            v_f = small.tile([1, R_CORE], F32, tag="v_f")
            nc.scalar.activation(v_f, lvk, mybir.ActivationFunctionType.Exp)

            vscr = nc.dram_tensor("vscr", [1, R_CORE], F32)
            nc.sync.dma_start(out=vscr[:], in_=v_f)
            v_pm = small.tile([128, RB], F32, tag="v_pm")
            nc.sync.dma_start(out=v_pm, in_=vscr.reshape((128, RB))[:])

            v_hl = small.tile([128, 2 * RB], BF16, tag="v_hl")
            nc.vector.tensor_copy(out=v_hl[:, 0 : 2 * RB : 2], in_=v_pm)
            vh_f = small.tile([128, RB], F32, tag="vh_f")
            nc.vector.tensor_copy(out=vh_f, in_=v_hl[:, 0 : 2 * RB : 2])
            nc.vector.tensor_tensor(
                out=v_hl[:, 1 : 2 * RB : 2], in0=v_pm, in1=vh_f, op=ts.subtract
            )

            # ---- matvec2: psum_dc += v_hl[j].T @ scatter(s_t[w, j]) ----
            for w in range(NW2):
                psum_dc = psd_pool.tile([2, W], F32, tag="psum_dc")
                for j in range(RB):
                    it = sti_pool.tile([128, NI2], I16, tag="sti")
                    dt_ = sti_pool.tile([128, NI2], BF16, tag="std")
                    nc.sync.dma_start(out=it, in_=st_idx[w, j])
                    nc.sync.dma_start(out=dt_, in_=st_dat[w, j])
                    sc = sts_pool.tile([128, W], BF16, tag="sts")
                    nc.gpsimd.local_scatter(
                        sc, dt_, it, channels=128, num_elems=W, num_idxs=NI2
                    )
                    scs = sts_pool.tile([128, W], BF16, tag="sts_s")
                    nc.vector.tensor_copy(out=scs, in_=sc)
                    for c in range(W // 512):
                        nc.tensor.matmul(
                            psum_dc[:, c * 512 : (c + 1) * 512],
                            v_hl[:, 2 * j : 2 * j + 2],
                            scs[:, c * 512 : (c + 1) * 512],
                            start=(j == 0),
                            stop=(j == RB - 1),
                        )
                st_out = stage_pool.tile([2, W], F32, tag="stage")
                nc.vector.tensor_copy(out=st_out, in_=psum_dc)
                nc.sync.dma_start(out=out[:, w * W : (w + 1) * W], in_=st_out)
    nc.compile()
    return nc


def _sparse_rows(mat, n_rows, width, pad):
    """mat: [n_rows, width] -> (idx [n_rows, pad] i16, val [n_rows, pad] bf16).

    Row-wise nonzero offsets (pad with -1) and values.  Raises if any row
    has more than `pad` nonzeros.
    """
    idx = np.full((n_rows, pad), -1, np.int16)
    val = np.zeros((n_rows, pad), np.float32)
    rr, cc = np.nonzero(mat)
    if len(rr):
        # rank of each entry within its row
        order = np.lexsort((cc, rr))
        rr, cc = rr[order], cc[order]
        starts = np.searchsorted(rr, np.arange(n_rows))
        rank = np.arange(len(rr)) - starts[rr]
        if rank.max() >= pad:
            raise ValueError(f"row nnz {rank.max() + 1} exceeds pad {pad}")
        idx[rr, rank] = cc.astype(np.int16)
        val[rr, rank] = mat[rr, cc]
    return idx, val.astype(BF16_NP)


def _prep_inputs(conc_balanced, S, balanced_species, unbalanced_species,
                 log_conc_unbalanced, log_kcat):
    """Host-side shard + sparse-encode prep (pure data movement / casts)."""
    in_maps = []
    xa_full = np.ones(N_SPECIES, dtype=np.float32)
    xb_full = np.zeros(N_SPECIES, dtype=np.float32)
    xa_full[np.asarray(balanced_species)] = np.asarray(conc_balanced)
    xb_full[np.asarray(unbalanced_species)] = np.asarray(log_conc_unbalanced)
    xa_pm = np.ascontiguousarray(xa_full.reshape(SB, 128).T)
    xb_pm = np.ascontiguousarray(xb_full.reshape(SB, 128).T)

    S = np.asarray(S)
    log_kcat = np.asarray(log_kcat)
    for c in range(N_CORES):
        r0 = c * R_CORE
        sl = S[:, r0 : r0 + R_CORE].astype(np.float32)       # [8192, 2048]
        # matvec1 stream: rows = (sb, w, p) -> species sb*128+p, window w
        sub = np.maximum(-sl, 0.0)                           # [8192, 2048]
        sub_rows = sub.reshape(SB, 128, NW1, W).transpose(0, 2, 1, 3)
        i1, v1 = _sparse_rows(
            sub_rows.reshape(-1, W), SB * NW1 * 128, W, NI1
        )
        # matvec2 stream: rows = (w, j, p) -> reaction r = p*16 + j,
        # species window w
        slT = sl.T                                           # [2048, 8192]
        st_rows = slT.reshape(128, RB, NW2, W).transpose(2, 1, 0, 3)
        i2, v2 = _sparse_rows(
            st_rows.reshape(-1, W), NW2 * RB * 128, W, NI2
        )
        kcat_pm = log_kcat[r0 : r0 + R_CORE].astype(np.float32).reshape(1, R_CORE)
        in_maps.append(
            {
                "su_idx": np.ascontiguousarray(i1.reshape(SB, NW1, 128, NI1)),
                "su_dat": np.ascontiguousarray(v1.reshape(SB, NW1, 128, NI1)),
                "st_idx": np.ascontiguousarray(i2.reshape(NW2, RB, 128, NI2)),
                "st_dat": np.ascontiguousarray(v2.reshape(NW2, RB, 128, NI2)),
                "xa": xa_pm,
                "xb": xb_pm,
                "kcat": np.ascontiguousarray(kcat_pm),
            }
        )
    return in_maps


def kernel(**inputs) -> np.ndarray:
    if "nc" not in _CACHE:
        _CACHE["nc"] = _build_nc()
    nc = _CACHE["nc"]
    in_maps = _prep_inputs(**inputs)
    res = run_bass_kernel_spmd(nc, in_maps, core_ids=list(range(N_CORES)))
    acc = np.zeros(N_SPECIES, dtype=np.float64)
    for c in range(N_CORES):
        o = res.results[c]["out"]
        acc += o[0].astype(np.float64) + o[1].astype(np.float64)
    return acc[:N_BAL].astype(np.float32)
